# revision 24
# baseline (speedup 1.0000x reference)
"""Trainium2 Bass kernel for additive-attention nn.Module.

Math: reference computes
    scores[b,i,j] = x[b,i,:]@W[0,:3] + key[b,j,:]@W[0,3:] + b0
    attn = softmax(scores, axis=j) ; out = attn @ value

softmax over j is shift-invariant, so the x- and bias-terms (constant in j)
cancel exactly: attn[b,i,j] = softmax_j(key[b,j,:]@W[0,3:]) independent of i.
Hence out[b,i,:] = sum_j p[b,j] * value[b,j,:]  (identical for every i).

The device computes only the unique rows out_row[b,:] = (sum_j e[b,j] *
value[b,j,:]) / s[b]; replicating them across the S1 axis is pure output
unsharding and happens on the host. This halves device HBM traffic vs
writing the full (B, S1, DV) tensor: per core it reads 8 MB of value and
writes 8 KB.

Kernel (data-parallel over batch, 8 batches/core on 8 cores):
  value SBUF layout: partition q holds rows j=8q..8q+7 (8 KB contiguous
  DMA per partition). key is pre-transposed on the host so the logits are
  computed directly in the matching layout eT[q, jj*8+b] = e[b, 8q+jj]:
  1. sk = key_r . w_k         (3 DVE fused mul-adds on [128, 64])
  2. eT = exp(sk)             (ACT, [128, 64])
  3. s via ones-matmul        (PE: [128,1]^T @ [128,64] -> [1,64]),
     tree-add over jj -> [1,8], reciprocal -> r_row (off critical path)
  4. out_row[b] = sum_jj eT[:, jj*8+b]^T @ v[b][:, jj*256:...]
     8 accumulating float32r matmuls [128,1]x[128,256] per batch (PE);
     float32r streams 1 col/cycle (plain fp32 matmul is 4x slower)
  5. normalize while copying PSUM->SBUF: o_sb[0, b*256:] = acc * r[b]
     (DVE/ACT alternating), single 8 KB DMA out at the end.

Value arrives as one ~1 MB DMA per batch, alternating between the two
HWDGE rings (SP / ACT) so descriptor generation is parallel and batch
data lands pipelined in batch order; per-batch matmuls overlap the
remaining stream. See inline comments for the ring-ordering and
tail-splitting details.
"""

import numpy as np
from contextlib import ExitStack

import concourse.bass as bass
import concourse.bacc as bacc
import concourse.mybir as mybir
from concourse import tile
from concourse.bass_utils import run_bass_kernel_spmd

B, S1, S2, DV = 64, 1024, 1024, 256
NCORES = 8
BPC = B // NCORES            # batches per core
NJ = S2 // 128               # j-slots per partition (8)
F32 = mybir.dt.float32
F32R = mybir.dt.float32r

_compiled = {}


def _build_nc():
    nc = bacc.Bacc("TRN2", target_bir_lowering=False, debug=False,
                   num_devices=NCORES)

    # ctrl[q, 0:192] = key_r (key_r[q, (jj*8+b)*3+f] = key[b, 8q+jj, f],
    # host pre-transposed); ctrl[q, 192:195] = w_k broadcast per partition
    ctrl_d = nc.dram_tensor("ctrl", [128, NJ * BPC * 3 + 3], F32,
                            kind="ExternalInput")
    val_d = nc.dram_tensor("value", [BPC, S2, DV], F32R, kind="ExternalInput")
    out_d = nc.dram_tensor("out", [1, BPC * DV], F32, kind="ExternalOutput")

    with tile.TileContext(nc) as tc, ExitStack() as ctx:
        const = ctx.enter_context(tc.tile_pool(name="const", bufs=1))
        sm = ctx.enter_context(tc.tile_pool(name="sm", bufs=1))
        vpool = ctx.enter_context(tc.tile_pool(name="v", bufs=BPC))
        ps_misc = ctx.enter_context(
            tc.tile_pool(name="ps_misc", bufs=1, space=bass.MemorySpace.PSUM))
        ps_acc = ctx.enter_context(
            tc.tile_pool(name="ps_acc", bufs=4, space=bass.MemorySpace.PSUM))

        # one control DMA, first on the ACT HWDGE ring
        ctrl_sb = const.tile([128, NJ * BPC * 3 + 3], F32)
        nc.scalar.dma_start(ctrl_sb[:], ctrl_d[:])
        kr_sb = ctrl_sb[:, 0:NJ * BPC * 3]
        wk_sb = ctrl_sb[:, NJ * BPC * 3:NJ * BPC * 3 + 3]
        ones_f = const.tile([128, 1], F32)
        nc.vector.memset(ones_f[:], 1.0)
        ones_sb = const.tile([128, 1], F32R)
        nc.vector.tensor_copy(ones_sb[:], ones_f[:])

        # value stream: one DMA per batch, alternating between the two
        # HWDGE rings (SP even / ACT odd) so descriptor generation runs in
        # parallel and batch data arrives pipelined in batch order.
        # partition q holds rows 8q..8q+7 of value[b] -> 8 KB contiguous
        # per partition. Ring-capacity backpressure makes later DIRECT2D
        # triggers stall on their sequencer, and anything queued behind
        # them in that engine's program order inherits the stall — exp
        # gates every matmul, so on the ACT ring only ctrl + v1 precede
        # exp; v3/v5/v7 triggers and the odd-batch copies interleave after
        # it. The first batch on each ring leads with a small piece
        # (engines start draining sooner: the ring TAIL is bumped per
        # DMA); the last batch on each ring trails with small pieces so
        # the post-arrival matmul work is ~1 matmul, not 8.
        W_ = NJ * DV
        v_tiles = []
        for _ in range(BPC):
            v_sb = vpool.tile([128, W_], F32R, tag="v_sb")
            v_tiles.append(v_sb)

        def dma_value(b):
            v_sb = v_tiles[b]
            v_src = val_d.ap()[b].rearrange("(q jj) d -> q (jj d)", q=128)
            eng = nc.sync if b % 2 == 0 else nc.scalar
            if b < 2:
                cuts = (0, W_ // 8, W_)
            elif b >= BPC - 2:
                cuts = (0, W_ // 2, 3 * W_ // 4, 7 * W_ // 8, W_)
            else:
                cuts = (0, W_)
            for lo, hi in zip(cuts[:-1], cuts[1:]):
                eng.dma_start(v_sb[:, lo:hi], v_src[:, lo:hi])

        dma_value(1)           # ACT ring: ctrl, v1, then exp below
        dma_value(0)           # SP ring starts streaming immediately
        dma_value(2)
        dma_value(4)

        # logits in transposed layout: sk[q, jj*8+b] = key_r . w_k
        k3 = kr_sb.rearrange("q (c f) -> q c f", f=3)
        sk0 = sm.tile([128, NJ * BPC], F32)
        sk1 = sm.tile([128, NJ * BPC], F32)
        eT = sm.tile([128, NJ * BPC], F32R)
        nc.vector.tensor_scalar_mul(sk0[:], k3[:, :, 0], wk_sb[:, 0:1])
        nc.vector.scalar_tensor_tensor(
            sk1[:], k3[:, :, 1], wk_sb[:, 1:2], sk0[:],
            op0=mybir.AluOpType.mult, op1=mybir.AluOpType.add)
        nc.vector.scalar_tensor_tensor(
            sk0[:], k3[:, :, 2], wk_sb[:, 2:3], sk1[:],
            op0=mybir.AluOpType.mult, op1=mybir.AluOpType.add)

        # eT = exp(sk)  (unnormalized softmax numerator, transposed layout)
        nc.scalar.activation(eT[:], sk0[:], mybir.ActivationFunctionType.Exp,
                             bias=0.0, scale=1.0)

        dma_value(3)           # ACT ring resumes after exp
        dma_value(6)           # SP ring
        dma_value(5)

        # softmax denominators: column-sums via ones-matmul, then reduce the
        # NJ j-slots per batch and invert. Off the matmul critical path.
        s_ps = ps_misc.tile([1, NJ * BPC], F32)
        nc.tensor.matmul(s_ps[:], ones_sb[:], eT[:], start=True, stop=True)
        s_sb = sm.tile([1, NJ * BPC], F32)
        nc.vector.tensor_copy(s_sb[:], s_ps[:])
        s_v = s_sb[:].rearrange("p (jj b) -> p jj b", b=BPC)
        t32 = sm.tile([1, 4 * BPC], F32)
        t32v = t32[:].rearrange("p (jj b) -> p jj b", b=BPC)
        nc.vector.tensor_add(t32v[:, 0:4, :], s_v[:, 0:4, :], s_v[:, 4:8, :])
        nc.vector.tensor_add(t32v[:, 0:2, :], t32v[:, 0:2, :], t32v[:, 2:4, :])
        nc.vector.tensor_add(t32v[:, 0:1, :], t32v[:, 0:1, :], t32v[:, 1:2, :])
        r_row = sm.tile([1, BPC], F32)
        nc.vector.reciprocal(r_row[:], t32[:, 0:BPC])

        # per-batch weighted sums on the PE: 8 accumulating float32r
        # matmuls [128,1] x [128,256] -> [1,256] per batch. Batch-serial
        # emission lets the strict-FIFO PE run ahead within a batch and
        # absorb inter-ring skew; only the final ring-pair (b6, b7) is
        # interleaved piece-by-piece — their pieces arrive in parallel on
        # the two rings at the very end, where a batch-serial order would
        # head-of-line-block b7's matmuls behind b6's last piece.
        o_sb = sm.tile([1, BPC * DV], F32)
        accs = {}

        def emit_mms(b, jj_lo, jj_hi):
            for jj in range(jj_lo, jj_hi):
                nc.tensor.matmul(
                    accs[b][:],
                    eT[:, jj * BPC + b:jj * BPC + b + 1],
                    v_tiles[b][:, jj * DV:(jj + 1) * DV],
                    start=(jj == 0), stop=(jj == NJ - 1),
                    skip_group_check=True)

        def emit_copy(b):
            dst = o_sb[:, b * DV:(b + 1) * DV]
            if b % 2 == 0:
                nc.vector.tensor_scalar_mul(dst, accs[b][:],
                                            r_row[:, b:b + 1])
            else:
                nc.scalar.mul(dst, accs[b][:], r_row[:, b:b + 1])

        for b in range(BPC - 2):
            accs[b] = ps_acc.tile([1, DV], F32, tag="acc", name="acc")
            emit_mms(b, 0, NJ)
            emit_copy(b)
            if b == 1:
                dma_value(7)   # last ACT-ring batch, after b1's copy

        accs[6] = ps_acc.tile([1, DV], F32, tag="acc", name="acc")
        accs[7] = ps_acc.tile([1, DV], F32, tag="acc", name="acc")
        for lo, hi in ((0, 4), (4, 6), (6, 7), (7, 8)):
            emit_mms(6, lo, hi)
            emit_mms(7, lo, hi)
        emit_copy(6)
        emit_copy(7)

        # out DMA issued from the ACT ring right after b7's copy (also on
        # ACT) — avoids a cross-engine semaphore hop in the tail
        nc.scalar.dma_start(out_d[:], o_sb[:])

    nc.compile()
    return nc


def _get_nc():
    if "nc" not in _compiled:
        _compiled["nc"] = _build_nc()
    return _compiled["nc"]


def _make_in_maps(key, value, W):
    key = np.ascontiguousarray(np.asarray(key, dtype=np.float32))
    value = np.asarray(value, dtype=np.float32)
    W = np.asarray(W, dtype=np.float32)
    wk128 = np.tile(W[0, 3:].reshape(1, 3), (128, 1))
    in_maps = []
    for c in range(NCORES):
        lo, hi = c * BPC, (c + 1) * BPC
        # key_r[q, jj, b, f] = key[b, 8q+jj, f]
        kc = key[lo:hi].reshape(BPC, 128, NJ, 3)
        keyr = kc.transpose(1, 2, 0, 3).reshape(128, NJ * BPC * 3)
        ctrl = np.ascontiguousarray(
            np.concatenate([keyr, wk128], axis=1))
        in_maps.append({
            "ctrl": ctrl,
            "value": np.ascontiguousarray(value[lo:hi]),
        })
    return in_maps


def kernel(x, key, value, W, b):
    nc = _get_nc()
    in_maps = _make_in_maps(key, value, W)
    res = run_bass_kernel_spmd(nc, in_maps, core_ids=list(range(NCORES)))
    rows = np.concatenate(
        [r["out"].reshape(BPC, DV) for r in res.results], axis=0)
    return np.ascontiguousarray(
        np.broadcast_to(rows[:, None, :], (B, S1, DV)))


def kernel_traced(x, key, value, W, b, **spmd_kwargs):
    """Like kernel() but returns (output, BassKernelResults) — for test.py."""
    nc = _get_nc()
    in_maps = _make_in_maps(key, value, W)
    res = run_bass_kernel_spmd(nc, in_maps, core_ids=list(range(NCORES)),
                               **spmd_kwargs)
    rows = np.concatenate(
        [r["out"].reshape(BPC, DV) for r in res.results], axis=0)
    out = np.ascontiguousarray(np.broadcast_to(rows[:, None, :], (B, S1, DV)))
    return out, res


# revision 27
# speedup vs baseline: 1.0648x; 1.0648x over previous
"""Trainium2 Bass kernel for additive-attention nn.Module.

Math: reference computes
    scores[b,i,j] = x[b,i,:]@W[0,:3] + key[b,j,:]@W[0,3:] + b0
    attn = softmax(scores, axis=j) ; out = attn @ value

softmax over j is shift-invariant, so the x- and bias-terms (constant in j)
cancel exactly: attn[b,i,j] = softmax_j(key[b,j,:]@W[0,3:]) independent of i.
Hence out[b,i,:] = sum_j p[b,j] * value[b,j,:]  (identical for every i).

The device computes only the unique rows out_row[b,:] = (sum_j e[b,j] *
value[b,j,:]) / s[b]; replicating them across the S1 axis is pure output
unsharding and happens on the host. This halves device HBM traffic vs
writing the full (B, S1, DV) tensor: per core it reads 8 MB of value and
writes 8 KB.

Kernel (data-parallel over batch, 8 batches/core on 8 cores). Batches are
processed in even/odd PAIRS: the host interleaves each pair's value rows
into [S2, 512] = [v_even | v_odd], so partition q holds rows j=8q..8q+7 of
the pair (16 KB contiguous DMA per partition), and ONE float32r matmul
[128,2] x [128,512] per j-slot computes both batches at once:
    psum[0, 0:256]   += e_even[j] * v_even[j]   (row 0, true)
    psum[1, 256:512] += e_odd[j]  * v_odd[j]    (row 1, true)
(the off-diagonal quadrants accumulate junk that is never read). This
halves the PE instruction count vs one matmul per (batch, slot) — the
LDWEIGHTS+MATMUL pair cost is instruction-bound at small N.

  1. sk = key_r . w_k       (3 DVE fused mul-adds on [128, 64]; key is
     host-pre-transposed into eT layout eT[q, jj*8+b] = e-col order)
  2. eT = exp(sk)           (ACT, [128, 64], written as float32r —
     float32r streams 1 col/cycle; plain fp32 matmul is 4x slower)
  3. s via ones-matmul      (PE: [128,2]^T @ [128,64] -> [2,64], both
     rows identical), tree-add over jj -> [2,8], reciprocal -> r2 on
     partitions 0 AND 1 so both copy flavors stay partition-aligned
  4. 8 accumulating pair-matmuls per pair (PE, float32r)
  5. normalize while evacuating PSUM: even rows via DVE to o_sb[0, :],
     odd rows via ACT to o_sb[1, :]; single 8 KB DMA out at the end
     (host un-interleaves the row order).

Value arrives as one ~2 MB DMA per pair, alternating between the two
HWDGE rings (SP / ACT) so descriptor generation is parallel and pair data
lands pipelined in order. Ring-capacity backpressure makes later DIRECT2D
triggers stall on their sequencer, and anything behind them in that
engine's program order inherits the stall — exp gates every matmul, so on
the ACT ring only ctrl + pair1 precede exp. The first pair on each ring
leads with a small piece (engines start draining sooner: the ring TAIL is
bumped per DMA); the last pair on each ring trails with small pieces, and
the two last pairs' matmuls are emitted piece-interleaved (the PE queue is
strict FIFO — batch-serial order would head-of-line-block one pair's
matmuls behind the other's last piece).
"""

import numpy as np
from contextlib import ExitStack

import concourse.bass as bass
import concourse.bacc as bacc
import concourse.mybir as mybir
from concourse import tile
from concourse.bass_utils import run_bass_kernel_spmd

B, S1, S2, DV = 64, 1024, 1024, 256
NCORES = 8
BPC = B // NCORES            # batches per core
NPAIR = BPC // 2             # batch pairs per core
NJ = S2 // 128               # j-slots per partition (8)
F32 = mybir.dt.float32
F32R = mybir.dt.float32r

_compiled = {}


def _build_nc():
    nc = bacc.Bacc("TRN2", target_bir_lowering=False, debug=False,
                   num_devices=NCORES)

    # ctrl[q, 0:192] = key_r (key_r[q, (jj*8+b)*3+f] = key[b, 8q+jj, f],
    # host pre-transposed); ctrl[q, 192:195] = w_k broadcast per partition
    # ctrl[q, 195:199] = pair-select mask (q<8: 1.0 iff q//2==k),
    # ctrl[q, 199:201] = parity-select (q<8: 1.0 iff q%2==p)
    ctrl_d = nc.dram_tensor("ctrl", [128, NJ * BPC * 3 + 9], F32,
                            kind="ExternalInput")
    # host-interleaved pairs: value[k, j, 0:256] = orig[2k, j],
    #                         value[k, j, 256:512] = orig[2k+1, j]
    val_d = nc.dram_tensor("value", [NPAIR, S2, 2 * DV], F32R,
                           kind="ExternalInput")
    out_d = nc.dram_tensor("out", [2, NPAIR * 2 * DV], F32,
                           kind="ExternalOutput")

    with tile.TileContext(nc) as tc, ExitStack() as ctx:
        const = ctx.enter_context(tc.tile_pool(name="const", bufs=1))
        sm = ctx.enter_context(tc.tile_pool(name="sm", bufs=1))
        vpool = ctx.enter_context(tc.tile_pool(name="v", bufs=NPAIR))
        ps_misc = ctx.enter_context(
            tc.tile_pool(name="ps_misc", bufs=1, space=bass.MemorySpace.PSUM))
        ps_acc = ctx.enter_context(
            tc.tile_pool(name="ps_acc", bufs=NPAIR,
                         space=bass.MemorySpace.PSUM))

        # one control DMA, first on the ACT HWDGE ring
        ctrl_sb = const.tile([128, NJ * BPC * 3 + 9], F32)
        nc.scalar.dma_start(ctrl_sb[:], ctrl_d[:])
        kr_sb = ctrl_sb[:, 0:NJ * BPC * 3]
        wk_sb = ctrl_sb[:, NJ * BPC * 3:NJ * BPC * 3 + 3]
        mask_sb = ctrl_sb[0:BPC, NJ * BPC * 3 + 3:NJ * BPC * 3 + 7]
        sel_sb = ctrl_sb[0:BPC, NJ * BPC * 3 + 7:NJ * BPC * 3 + 9]
        ones_f = const.tile([128, 2], F32)
        nc.vector.memset(ones_f[:], 1.0)
        ones_sb = const.tile([128, 2], F32R)
        nc.vector.tensor_copy(ones_sb[:], ones_f[:])

        W2 = NJ * 2 * DV     # floats per partition per pair (4096)
        v_tiles = []
        for _ in range(NPAIR):
            v_sb = vpool.tile([128, W2], F32R, tag="v_sb")
            v_tiles.append(v_sb)

        def dma_pair(k):
            v_sb = v_tiles[k]
            v_src = val_d.ap()[k].rearrange("(q jj) d -> q (jj d)", q=128)
            eng = nc.sync if k % 2 == 0 else nc.scalar
            if k < 2:
                cuts = (0, W2 // 8, W2)                      # 1 slot, 7
            else:
                cuts = (0, W2 // 2, 3 * W2 // 4,
                        7 * W2 // 8, W2)                     # 4,2,1,1
            for lo, hi in zip(cuts[:-1], cuts[1:]):
                eng.dma_start(v_sb[:, lo:hi], v_src[:, lo:hi])

        dma_pair(1)            # ACT ring: ctrl, pair1, then exp below
        dma_pair(0)            # SP ring starts streaming immediately
        dma_pair(2)

        # logits in transposed layout: sk[q, jj*8+b] = key_r . w_k
        k3 = kr_sb.rearrange("q (c f) -> q c f", f=3)
        sk0 = sm.tile([128, NJ * BPC], F32)
        sk1 = sm.tile([128, NJ * BPC], F32)
        eT = sm.tile([128, NJ * BPC], F32R)
        nc.vector.tensor_scalar_mul(sk0[:], k3[:, :, 0], wk_sb[:, 0:1])
        nc.vector.scalar_tensor_tensor(
            sk1[:], k3[:, :, 1], wk_sb[:, 1:2], sk0[:],
            op0=mybir.AluOpType.mult, op1=mybir.AluOpType.add)
        nc.vector.scalar_tensor_tensor(
            sk0[:], k3[:, :, 2], wk_sb[:, 2:3], sk1[:],
            op0=mybir.AluOpType.mult, op1=mybir.AluOpType.add)

        # eT = exp(sk)  (unnormalized softmax numerator, transposed layout)
        nc.scalar.activation(eT[:], sk0[:], mybir.ActivationFunctionType.Exp,
                             bias=0.0, scale=1.0)

        dma_pair(3)            # ACT ring resumes after exp

        # softmax denominators on partitions 0 AND 1: column-sums via
        # ones-matmul (both output rows identical), tree-reduce the NJ
        # j-slots per batch, invert. Off the matmul critical path.
        s_ps = ps_misc.tile([2, NJ * BPC], F32)
        nc.tensor.matmul(s_ps[:], ones_sb[:], eT[:], start=True, stop=True)
        s_sb = sm.tile([2, NJ * BPC], F32)
        nc.vector.tensor_copy(s_sb[:], s_ps[:])
        s_v = s_sb[:].rearrange("p (jj b) -> p jj b", b=BPC)
        t32 = sm.tile([2, 4 * BPC], F32)
        t32v = t32[:].rearrange("p (jj b) -> p jj b", b=BPC)
        nc.vector.tensor_add(t32v[:, 0:4, :], s_v[:, 0:4, :], s_v[:, 4:8, :])
        nc.vector.tensor_add(t32v[:, 0:2, :], t32v[:, 0:2, :], t32v[:, 2:4, :])
        nc.vector.tensor_add(t32v[:, 0:1, :], t32v[:, 0:1, :], t32v[:, 1:2, :])

        # paired reciprocals rp[p, k] = 1/s[2k+p], built with base-0 ops
        # only (PSUM/engine ops may not start at partition 1): transpose s
        # to partitions, invert, mask per pair, and select parity per
        # output partition with a tiny matmul.
        sT_ps = ps_misc.tile([BPC, 1], F32, name="sT_ps")
        nc.tensor.transpose(sT_ps[:], t32[0:1, 0:BPC], ones_f[0:1, 0:1])
        sT = sm.tile([BPC, 1], F32)
        nc.vector.tensor_copy(sT[:], sT_ps[:])
        rT = sm.tile([BPC, 1], F32)
        nc.vector.reciprocal(rT[:], sT[:])
        rdiag = sm.tile([BPC, NPAIR], F32)
        nc.vector.tensor_scalar_mul(rdiag[:], mask_sb, rT[:])
        rp_ps = ps_misc.tile([2, NPAIR], F32, name="rp_ps")
        nc.tensor.matmul(rp_ps[:], sel_sb, rdiag[:], start=True, stop=True)
        rp = sm.tile([2, NPAIR], F32)
        nc.vector.tensor_copy(rp[:], rp_ps[:])

        # pair weighted sums on the PE: 8 accumulating float32r matmuls
        # [128,2] x [128,512] -> [2,512] per pair. acc tiles are allocated
        # full-bank ([128,512]) so no two pairs' interleaved accumulation
        # groups share a PSUM bank (start=True clears has_written for the
        # whole bank).
        o_sb = sm.tile([2, NPAIR * 2 * DV], F32)
        accs = {}

        def emit_mms(k, jj_lo, jj_hi):
            for jj in range(jj_lo, jj_hi):
                nc.tensor.matmul(
                    accs[k][0:2, :],
                    eT[:, jj * BPC + 2 * k:jj * BPC + 2 * k + 2],
                    v_tiles[k][:, jj * 2 * DV:(jj + 1) * 2 * DV],
                    start=(jj == 0), stop=(jj == NJ - 1),
                    skip_group_check=True)

        def emit_copies(k):
            # one base-0 op evacuates both rows (junk quadrants included;
            # the host slices the true ones); rp is a per-partition scalar
            dst = o_sb[0:2, k * 2 * DV:(k + 1) * 2 * DV]
            if k % 2 == 0:
                nc.vector.tensor_scalar_mul(dst, accs[k][0:2, :],
                                            rp[:, k:k + 1])
            else:
                nc.scalar.mul(dst, accs[k][0:2, :], rp[:, k:k + 1])

        for k in range(NPAIR - 2):
            accs[k] = ps_acc.tile([128, 2 * DV], F32, tag="acc", name="acc")
            emit_mms(k, 0, NJ)
            emit_copies(k)

        # the two last pairs (one per ring) arrive in parallel at the very
        # end: interleave their matmuls piece-by-piece
        accs[2] = ps_acc.tile([128, 2 * DV], F32, tag="acc", name="acc")
        accs[3] = ps_acc.tile([128, 2 * DV], F32, tag="acc", name="acc")
        for lo, hi in ((0, 4), (4, 6), (6, 7), (7, 8)):
            emit_mms(2, lo, hi)
            emit_mms(3, lo, hi)
        emit_copies(2)
        emit_copies(3)

        # out DMA issued from the ACT ring right after the last copy (also
        # on ACT) — avoids a cross-engine semaphore hop in the tail
        nc.scalar.dma_start(out_d[:], o_sb[:])

    nc.compile()
    return nc


def _get_nc():
    if "nc" not in _compiled:
        _compiled["nc"] = _build_nc()
    return _compiled["nc"]


def _make_in_maps(key, value, W):
    key = np.ascontiguousarray(np.asarray(key, dtype=np.float32))
    value = np.asarray(value, dtype=np.float32)
    W = np.asarray(W, dtype=np.float32)
    wk128 = np.tile(W[0, 3:].reshape(1, 3), (128, 1))
    in_maps = []
    for c in range(NCORES):
        lo, hi = c * BPC, (c + 1) * BPC
        # key_r[q, jj, b, f] = key[b, 8q+jj, f]
        kc = key[lo:hi].reshape(BPC, 128, NJ, 3)
        keyr = kc.transpose(1, 2, 0, 3).reshape(128, NJ * BPC * 3)
        mask48 = np.zeros((128, 4), dtype=np.float32)
        sel2 = np.zeros((128, 2), dtype=np.float32)
        for q in range(BPC):
            mask48[q, q // 2] = 1.0
            sel2[q, q % 2] = 1.0
        ctrl = np.ascontiguousarray(
            np.concatenate([keyr, wk128, mask48, sel2], axis=1))
        # interleave even/odd batch pairs along the feature axis
        vpair = np.ascontiguousarray(
            value[lo:hi].reshape(NPAIR, 2, S2, DV).transpose(0, 2, 1, 3)
            .reshape(NPAIR, S2, 2 * DV))
        in_maps.append({
            "ctrl": ctrl,
            "value": vpair,
        })
    return in_maps


def _gather(res):
    rows = np.empty((B, DV), dtype=np.float32)
    for c, r in enumerate(res.results):
        o = r["out"].reshape(2, NPAIR, 2, DV)
        rows[c * BPC:(c + 1) * BPC:2] = o[0, :, 0]     # even: row 0, lo half
        rows[c * BPC + 1:(c + 1) * BPC:2] = o[1, :, 1]  # odd: row 1, hi half
    return np.ascontiguousarray(
        np.broadcast_to(rows[:, None, :], (B, S1, DV)))


def kernel(x, key, value, W, b):
    nc = _get_nc()
    in_maps = _make_in_maps(key, value, W)
    res = run_bass_kernel_spmd(nc, in_maps, core_ids=list(range(NCORES)))
    return _gather(res)


def kernel_traced(x, key, value, W, b, **spmd_kwargs):
    """Like kernel() but returns (output, BassKernelResults) — for test.py."""
    nc = _get_nc()
    in_maps = _make_in_maps(key, value, W)
    res = run_bass_kernel_spmd(nc, in_maps, core_ids=list(range(NCORES)),
                               **spmd_kwargs)
    return _gather(res), res
